# revision 23
# baseline (speedup 1.0000x reference)
"""Single-head attention kernel for Trainium2 (8 NeuronCores, SPMD).

Problem: x[4,4096,1024] f32, padding_mask[4,1,4096] i32, Wk/Wq/Wv[64,1024] f32.
  k/q/v = x @ W.T ; wei = softmax(mask(q k^T / 8)) ; out = wei @ v  -> [4,4096,64]

Structural wins over the naive mapping:
  * Compaction: masked key columns contribute 0 weight and masked query rows
    output exactly 0, and the key mask equals the query mask.  The host gathers
    each batch's ~2048 unmasked rows, pads to NU (multiple of 256), and the
    device computes attention only over the compacted set.  Pad rows are zero,
    so their v-contribution is 0; the softmax denominator is computed via an
    extra "ones" column in v that the host zeroes for pad keys -- no masking
    bias is needed anywhere on the device.
  * Host pre-transposes x to xT[bf16] so the device does no PE transposes of x
    and HBM traffic is halved (bf16).
  * All matmul operands are bf16 (1 cycle/row, FWL weight loads); PSUM stays
    f32.
  * Sharding: 2 cores per batch; core half h computes compacted queries
    [r, r+qL) where r = nU-qL for h=1 (keys rotated by r, which is an exact
    permutation invariance), so no computed query row is wasted on padding.
  * Projections pack [k|v] into one 128-col stationary; odd t-blocks use
    [v|k] so kT lands at PSUM partitions 64:128 for them.  That gives score
    chunks at both partition bases, enabling ROW-TILED score matmuls: two
    K=64 matmuls run concurrently in the PE array (rows 0:63 and 64:127).
    q uses a duplicated [wq|wq] stationary so qT is born replicated on both
    partition halves.
"""

import sys

if "/opt/trn_rl_repo" not in sys.path:
    sys.path.insert(0, "/opt/trn_rl_repo")

import numpy as np
import ml_dtypes

import concourse.bass as bass
import concourse.mybir as mybir
import concourse.tile as tile
from concourse import bacc
from concourse.bass_utils import run_bass_kernel_spmd

F32 = mybir.dt.float32
BF16 = mybir.dt.bfloat16
BF_NP = ml_dtypes.bfloat16
B, T, C, H = 4, 4096, 1024, 64
NCC = C // 128  # 8 c-chunks


def _blocks(total, step):
    out, t0 = [], 0
    while t0 < total:
        out.append((t0, min(step, total - t0)))
        t0 += step
    return out


def build_nc(NU, parity=True, force_singles=False):
    qL = NU // 2
    NKC = NU // 128
    nc = bacc.Bacc("TRN2", target_bir_lowering=False, debug=False, num_devices=8)

    xT_d = nc.dram_tensor("xt", [C, NU], BF16, kind="ExternalInput")
    wpack_d = nc.dram_tensor("wpack", [128, 3, NCC, 128], BF16, kind="ExternalInput")
    fpack_d = nc.dram_tensor("fpack", [128, 192 + NKC], F32, kind="ExternalInput")
    out_d = nc.dram_tensor("out", [qL, H], F32, kind="ExternalOutput")

    tb_blocks = _blocks(NU, 512)
    qb_blocks = _blocks(qL, 512)

    # key-chunk -> (owning t-block, psum partition base of kT for that block)
    kc_tb = []
    for i, (t0, tbs) in enumerate(tb_blocks):
        for _ in range(tbs // 128):
            kc_tb.append(i)
    base_k = [0 if (kc_tb[kc] % 2 == 0 or not parity) else 64 for kc in range(NKC)]
    evens = [kc for kc in range(NKC) if base_k[kc] == 0]
    odds = [kc for kc in range(NKC) if base_k[kc] == 64]
    npairs = 0 if force_singles else min(len(evens), len(odds))
    pairs = list(zip(evens[:npairs], odds[:npairs]))
    singles = evens[npairs:] + odds[npairs:]

    with tile.TileContext(nc) as tc:
        with (
            tc.tile_pool(name="const", bufs=1) as const,
            tc.tile_pool(name="persist", bufs=1) as persist,
            tc.tile_pool(name="expp", bufs=3) as expp,
            tc.tile_pool(name="osb", bufs=3) as osb,
            tc.tile_pool(name="small", bufs=4) as small,
            tc.tile_pool(name="psP", bufs=2, space=bass.MemorySpace.PSUM) as psP,
            tc.tile_pool(name="psS", bufs=2, space=bass.MemorySpace.PSUM) as psS,
            tc.tile_pool(name="psO", bufs=2, space=bass.MemorySpace.PSUM) as psO,
        ):
            # ---- constants (2 packed DMAs) ----
            wpack_sb = const.tile([128, 3, NCC, 128], BF16)
            fpack_sb = const.tile([128, 192 + NKC], F32)
            nc.sync.dma_start(out=wpack_sb, in_=wpack_d.ap())
            nc.scalar.dma_start(out=fpack_sb, in_=fpack_d.ap())
            wkv_sb, wvk_sb, wqd_sb = wpack_sb[:, 0], wpack_sb[:, 1], wpack_sb[:, 2]
            identd_sb = fpack_sb[:, 0:64]
            identf_sb = fpack_sb[:, 64:192]
            onesk_sb = fpack_sb[:, 192 : 192 + NKC]

            # ---- persistent intermediates ----
            xT_sb = persist.tile([128, NCC, NU], BF16)
            kT_rep = persist.tile([128, NU], BF16)
            qT_rep = persist.tile([128, qL], BF16)
            vT_sb = persist.tile([128, NU], F32)
            v_sb = persist.tile([128, NKC, H + 1], BF16)
            out_acc = persist.tile([128, qL // 128, H], F32)
            nc.vector.tensor_copy(v_sb[:, :, H], onesk_sb)

            # x: two half-slabs per c-chunk (few, large DMAs -- the DMA
            # issue rate, not bandwidth, limits the lead-in), alternating
            # between the two HWDGE queues
            dma_engines = [nc.sync, nc.scalar]
            cuts = [0, 512, NU // 2, NU]
            di = 0
            for ci in range(len(cuts) - 1):
                h0, h1 = cuts[ci], cuts[ci + 1]
                for cc in range(NCC):
                    dma_engines[di % 2].dma_start(
                        out=xT_sb[:, cc, h0:h1],
                        in_=xT_d.ap()[cc * 128 : (cc + 1) * 128, h0:h1],
                    )
                    di += 1

            # ===== Phase 1 + interleaved Phase 2 (qb0) =====
            # Score/exp/AV "waves" for the first query block are emitted as
            # soon as their key chunks are projected, so ACT exp work overlaps
            # the remaining projections and the PE never idles long enough to
            # re-throttle (HAM).
            chunk_groups = [list(p) for p in pairs] + [[kc] for kc in singles]
            # group ready after t-block: max owning tb over its chunks
            group_ready = [max(kc_tb[kc] for kc in kcs) for kcs in chunk_groups]

            qstate = {}  # qb index -> (oT_ps, groups emitted, av cursor)

            def emit_tb(i, t0, tbs):
                even = i % 2 == 0
                w_sb = wkv_sb if even else wvk_sb
                bk, bv = (0, 64) if even else (64, 0)
                kv_ps = psP.tile([128, 512], F32, tag="p", name="kv_ps")
                for cc in range(NCC):
                    nc.tensor.matmul(
                        kv_ps[:, :tbs],
                        w_sb[:, cc, :],
                        xT_sb[:, cc, t0 : t0 + tbs],
                        start=(cc == 0),
                        stop=(cc == NCC - 1),
                    )
                nc.vector.tensor_copy(
                    kT_rep[bk : bk + 64, t0 : t0 + tbs], kv_ps[bk : bk + 64, :tbs]
                )
                nc.vector.tensor_copy(
                    vT_sb[bv : bv + 64, t0 : t0 + tbs], kv_ps[bv : bv + 64, :tbs]
                )
                if t0 < qL:
                    qbs = min(tbs, qL - t0)
                    q_ps = psP.tile([128, 512], F32, tag="p", name="q_ps")
                    for cc in range(NCC):
                        nc.tensor.matmul(
                            q_ps[:, :qbs],
                            wqd_sb[:, cc, :],
                            xT_sb[:, cc, t0 : t0 + qbs],
                            start=(cc == 0),
                            stop=(cc == NCC - 1),
                        )
                    nc.vector.tensor_copy(qT_rep[:, t0 : t0 + qbs], q_ps[:, :qbs])
                for j in range(tbs // 128):
                    kc = t0 // 128 + j
                    vt_ps = psP.tile([128, 512], F32, tag="p", name="vt_ps")
                    nc.tensor.transpose(
                        vt_ps[:, 0:H],
                        vT_sb[bv : bv + 64, kc * 128 : (kc + 1) * 128],
                        identd_sb[bv : bv + 64, :],
                    )
                    nc.vector.tensor_copy(v_sb[:, kc, 0:H], vt_ps[:, 0:H])

            def emit_groups(qi, glo, ghi):
                q0, qbs = qb_blocks[qi]
                cap = 2 * (512 // qbs)
                if qi not in qstate:
                    oT_ps = psO.tile([H + 1, 512], F32, name="oT_ps")
                    # [oT_ps, closed chunk list (kc, exp, off), open tile, av cursor]
                    qstate[qi] = [oT_ps, [], None, 0]
                st = qstate[qi]

                def close_tile():
                    tile = st[2]
                    if tile is None:
                        return
                    sT_ps, exp_sb, kclist = tile
                    lo_n = (len(kclist) + 1) // 2
                    hi_n = len(kclist) // 2
                    # one ACT per PSUM bank half: the low half finishes
                    # early so the first AV matmul unblocks ~0.7us sooner
                    nc.scalar.activation(
                        exp_sb[:, 0 : lo_n * qbs],
                        sT_ps[:, 0 : lo_n * qbs],
                        mybir.ActivationFunctionType.Exp,
                        scale=0.125,
                    )
                    if hi_n:
                        nc.scalar.activation(
                            exp_sb[:, 512 : 512 + hi_n * qbs],
                            sT_ps[:, 512 : 512 + hi_n * qbs],
                            mybir.ActivationFunctionType.Exp,
                            scale=0.125,
                        )
                    # AV order: low-half chunks first (their exp lands first)
                    for kc, off in sorted(kclist, key=lambda t: t[1]):
                        st[1].append((kc, exp_sb, off))
                    st[2] = None

                for g in range(glo, ghi):
                    kcs = chunk_groups[g]
                    if st[2] is not None and len(st[2][2]) + len(kcs) > cap:
                        close_tile()
                        _drain_av(qi)
                    if st[2] is None:
                        sT_ps = psS.tile([128, 1024], F32, tag="s", name="sT_ps")
                        exp_sb = expp.tile([128, 1024], BF16, name="exp_sb")
                        st[2] = [sT_ps, exp_sb, []]
                    sT_ps, exp_sb, kclist = st[2]
                    for kc in kcs:
                        slot = len(kclist)
                        off = (slot % 2) * 512 + (slot // 2) * qbs
                        bkc = base_k[kc]
                        nc.tensor.matmul(
                            sT_ps[:, off : off + qbs],
                            kT_rep[bkc : bkc + 64, kc * 128 : (kc + 1) * 128],
                            qT_rep[bkc : bkc + 64, q0 : q0 + qbs],
                            start=True,
                            stop=True,
                        )
                        kclist.append((kc, off))

            def _drain_av(qi):
                q0, qbs = qb_blocks[qi]
                st = qstate[qi]
                while st[3] < len(st[1]):
                    kc, exp_sb, off = st[1][st[3]]
                    nc.tensor.matmul(
                        st[0][:, 0:qbs],
                        v_sb[:, kc, :],
                        exp_sb[:, off : off + qbs],
                        start=(st[3] == 0),
                        stop=(st[3] == NKC - 1),
                    )
                    st[3] += 1

            def finish_qb(qi):
                q0, qbs = qb_blocks[qi]
                st = qstate[qi]
                # close the open tile and drain remaining AV work
                if st[2] is not None:
                    sT_ps, exp_sb, kclist = st[2]
                    lo_n = (len(kclist) + 1) // 2
                    hi_n = len(kclist) // 2
                    nc.scalar.activation(
                        exp_sb[:, 0 : lo_n * qbs],
                        sT_ps[:, 0 : lo_n * qbs],
                        mybir.ActivationFunctionType.Exp,
                        scale=0.125,
                    )
                    if hi_n:
                        nc.scalar.activation(
                            exp_sb[:, 512 : 512 + hi_n * qbs],
                            sT_ps[:, 512 : 512 + hi_n * qbs],
                            mybir.ActivationFunctionType.Exp,
                            scale=0.125,
                        )
                    for kc, off in sorted(kclist, key=lambda t: t[1]):
                        st[1].append((kc, exp_sb, off))
                    st[2] = None
                _drain_av(qi)
                oT_sb = osb.tile([H + 1, 512], F32, name="oT_sb")
                nc.vector.tensor_copy(oT_sb[:, :qbs], st[0][:, :qbs])
                return oT_sb

            ngroups = len(chunk_groups)
            waves = []  # (after_tb, glo, ghi) for qb0
            prev = 0
            for i in range(len(tb_blocks)):
                ghi = sum(1 for r in group_ready if r <= i)
                if ghi > prev:
                    waves.append((i, prev, ghi))
                    prev = ghi

            oT_sbs = []
            wi = 0
            for i, (t0, tbs) in enumerate(tb_blocks):
                emit_tb(i, t0, tbs)
                while wi < len(waves) and waves[wi][0] == i:
                    emit_groups(0, waves[wi][1], waves[wi][2])
                    wi += 1
            oT_sbs.append(finish_qb(0))
            for qi in range(1, len(qb_blocks)):
                emit_groups(qi, 0, ngroups)
                oT_sbs.append(finish_qb(qi))

            # ===== deferred epilogues: transpose back + 1/denominator =====
            for qi, (q0, qbs) in enumerate(qb_blocks):
                oT_sb = oT_sbs[qi]
                for qs in range(qbs // 128):
                    qt = q0 // 128 + qs
                    ot_ps = psP.tile([128, 512], F32, tag="p", name="ot_ps")
                    nc.tensor.transpose(
                        ot_ps[:, 0 : H + 1],
                        oT_sb[:, qs * 128 : (qs + 1) * 128],
                        identf_sb[: H + 1, : H + 1],
                    )
                    recip_sb = small.tile([128, 1], F32, name="recip_sb")
                    nc.vector.reciprocal(recip_sb, ot_ps[:, H : H + 1])
                    nc.vector.tensor_scalar_mul(
                        out_acc[:, qt, :], ot_ps[:, 0:H], recip_sb
                    )
            nc.sync.dma_start(
                out=out_d.ap().rearrange("(n p) h -> p n h", p=128), in_=out_acc
            )

    nc.compile()
    return nc



_NC_CACHE = {}


def _get_nc(NU):
    if NU not in _NC_CACHE:
        _NC_CACHE[NU] = build_nc(NU)
    return _NC_CACHE[NU]


def make_in_maps(x, padding_mask, Wk, Wq, Wv):
    x = np.asarray(x)
    padding_mask = np.asarray(padding_mask)
    Wk, Wq, Wv = (np.asarray(w, np.float32) for w in (Wk, Wq, Wv))

    idxs = [np.nonzero(padding_mask[b, 0])[0] for b in range(B)]
    nUs = [len(ix) for ix in idxs]
    NU = max(256, -(-max(nUs) // 256) * 256)
    qL = NU // 2
    NKC = NU // 128

    def wt(w):  # [64,1024] -> [128, NCC, 64]: wt[p, cc, h] = w[h, cc*128+p]
        return w.T.reshape(NCC, 128, H).transpose(1, 0, 2)

    wk, wq, wv = wt(Wk), wt(Wq), wt(Wv)
    wkv = np.concatenate([wk, wv], axis=2)
    wvk = np.concatenate([wv, wk], axis=2)
    wqd = np.concatenate([wq, wq], axis=2)
    wpack = np.ascontiguousarray(
        np.stack([wkv, wvk, wqd], axis=1).astype(BF_NP)
    )
    identd = np.vstack([np.eye(64, dtype=np.float32)] * 2)
    identf = np.eye(128, dtype=np.float32)

    in_maps = []
    for b in range(B):
        ix, nU = idxs[b], nUs[b]
        x_u = np.zeros((NU, C), np.float32)
        x_u[:nU] = x[b][ix]
        xT0 = np.ascontiguousarray(x_u.T).astype(BF_NP)
        ones = np.zeros(NU, np.float32)
        ones[:nU] = 1.0
        for h in range(2):
            r = max(nU - qL, 0) if h else 0
            if r:
                xT = np.ascontiguousarray(np.roll(xT0, -r, axis=1))
                ones_r = np.roll(ones, -r)
            else:
                xT, ones_r = xT0, ones
            onesk = ones_r.reshape(NKC, 128).T
            fpack = np.ascontiguousarray(
                np.concatenate([identd, identf, onesk], axis=1).astype(np.float32)
            )
            in_maps.append({"xt": xT, "wpack": wpack, "fpack": fpack})
    return NU, idxs, nUs, in_maps


def kernel(x, padding_mask, Wk, Wq, Wv):
    NU, idxs, nUs, in_maps = make_in_maps(x, padding_mask, Wk, Wq, Wv)
    qL = NU // 2
    nc = _get_nc(NU)
    res = run_bass_kernel_spmd(nc, in_maps, core_ids=list(range(8)), trace=False)
    out = np.zeros((B, T, H), dtype=np.float32)
    for b in range(B):
        ix, nU = idxs[b], nUs[b]
        res0 = res.results[2 * b]["out"]
        res1 = res.results[2 * b + 1]["out"]
        n0 = min(qL, nU)
        out[b, ix[0:n0]] = res0[0:n0]
        r = max(nU - qL, 0)
        out[b, ix[r:nU]] = res1[0 : nU - r]
    return out
